# revision 1
# baseline (speedup 1.0000x reference)
"""AO layer kernel for Trainium2 (8 NeuronCores, data-parallel over walkers).

Math: out[b,n,a] = ang(a, r) * rad(a, r),  r = pos[b,n] - centers[a]
  rad = sum_p coeffs[a,p] * exp(-exps[a,p] * |r|^2)
  ang = prod_c r_c^powers[a,c],  powers in {0,1,2}

Design (per core, i = flattened (b,n) walker-electron index, I=2048):
  All inputs ship as ONE packed DRAM tensor, segment order chosen so 4
  chunked DMAs land just-in-time (HWDGE issue is serialized); the full
  21-row basis [xh,xl,1,0, sqh,sql, sqh,xh,1] is built on the HOST
  (bf16 hi/lo splits of x and x^2 in f64), zero-padded to k=128.
  z[(a,p), i]   = W1^T R    (PE bf16, 12 [128,512] passes per i-tile)
  E = exp(z)                (ACT, [128,1024] tiles, bf16 out)
  rad[a, i]     = S^T E     (PE bf16 accumulate, 12 passes per i-tile)
  p_c[a, i]     = Q_c^T R   (PE bf16, 6 passes per i-tile)
  ang = px*py*pz            (DVE; GPSIMD cannot touch PSUM on HW)
  out[a, i]     = ang*rad   (DVE), DMA straight to DRAM in [A, I]
  layout — the (i, a) transpose happens on the host during unshard.
  PSUM: zp 2x[128,1024] (4 banks) + shared ang/rad pool 4x[128,512]
  (4 banks) so ang of i-tile N+1 pipelines under the radial of N.
  The body is paced by twin floors: PE 30x[128,512] matmul passes per
  i-tile (~25.6us warm) and ACT exp over A*P*I elements (~25.1us);
  both run ~95%+ dense between a ~3.9us DMA-latency head and a ~5us
  exp->rad->mul->DMA tail.
"""

import numpy as np
import ml_dtypes

B, NEL, A, P = 512, 32, 256, 6
NCORES = 8
BS = B // NCORES          # 64 walkers per core
I = BS * NEL              # 2048 (b,n) pairs per core
ITILE = 512
NIT = I // ITILE          # 4 i-tiles
RT = (A * P) // 128       # 12 r-tiles of 128 (a,p) rows
K7 = 7

# --- packed input layout (columns of the [128, PK] bf16 "pk" tensor) ---
PK_RR0 = 0                # rr it0            [0:512)
PK_W1A = 512              # w1 r-tiles 0-1    [512:768)
PK_Q = 768                # q                 [768:1536)
PK_W1B = 1536             # w1 r-tiles 2-3    [1536:1792)
PK_SA = 1792              # s r-tiles 0-1     [1792:2048)
PK_SB = 2048              # s r-tiles 2-5     [2048:2560)
PK_W1C = 2560             # w1 r-tiles 4-11   [2560:3584)
PK_SC = 3584              # s r-tiles 6-11    [3584:4352)
PK_RR1 = 4352             # rr it1-3          [4352:5888)
PK = 5888
PK_CHUNKS = [0, 768, 2048, 3584, PK]


def _w1_col(rt):
    if rt < 2:
        return PK_W1A + 128 * rt
    if rt < 4:
        return PK_W1B + 128 * (rt - 2)
    return PK_W1C + 128 * (rt - 4)


def _s_col(rt):
    if rt < 2:
        return PK_SA + 128 * rt
    if rt < 6:
        return PK_SB + 128 * (rt - 2)
    return PK_SC + 128 * (rt - 6)


def _rr_col(it):
    return PK_RR0 if it == 0 else PK_RR1 + 512 * (it - 1)


_CACHE = {}


def _bf(v):
    return np.asarray(v, np.float64).astype(ml_dtypes.bfloat16)


def _split21(w):
    """bf16 hi/lo split of [7, C] weights, laid out to match the basis
    rows [xh,xl,1,0, sqh,sql, sqh2,xh2,1] and zero-padded to k=128."""
    wh = _bf(w).astype(np.float64)
    wl = _bf(np.asarray(w, np.float64) - wh)
    wh = _bf(wh)
    z1 = np.zeros((1, w.shape[1]), ml_dtypes.bfloat16)
    out = np.concatenate([
        wh[3:6], wh[3:6], wh[6:7], z1,        # xh, xl, 1, 0
        wh[0:3], wh[0:3],                     # sqh, sql
        wl[0:3], wl[3:6], wl[6:7],            # sqh2, xh2, 1
        np.zeros((128 - 21, w.shape[1]), ml_dtypes.bfloat16),
    ], axis=0)
    return np.ascontiguousarray(out)


def _build_nc():
    import concourse.bass as bass
    import concourse.bacc as bacc
    import concourse.tile as tile
    import concourse.mybir as mybir

    f32 = mybir.dt.float32
    bf16 = mybir.dt.bfloat16
    EXP = mybir.ActivationFunctionType.Exp
    PSUM = bass.MemorySpace.PSUM

    nc = bacc.Bacc("TRN2", target_bir_lowering=False, debug=False,
                   num_devices=NCORES)

    pk_d = nc.declare_dram_parameter("pk", [128, PK], bf16, isOutput=False)
    out_d = nc.declare_dram_parameter("out", [A, I], f32, isOutput=True)

    with tile.TileContext(nc) as tc:
        with (
            tc.tile_pool(name="const", bufs=1) as const,
            tc.tile_pool(name="zp", bufs=2, space=PSUM) as zp,
            tc.tile_pool(name="pr", bufs=4, space=PSUM) as pr,
            tc.tile_pool(name="ep", bufs=8) as ep,
            tc.tile_pool(name="mid", bufs=6) as mid,
            tc.tile_pool(name="angp", bufs=6) as angp,
            tc.tile_pool(name="op", bufs=6) as op,
        ):
            pk_sb = const.tile([128, PK], bf16)
            for c0, c1 in zip(PK_CHUNKS[:-1], PK_CHUNKS[1:]):
                nc.sync.dma_start(pk_sb[:, c0:c1], pk_d[:, c0:c1])

            def mm(out_ap, lhs_ap, rhs_ap, start=True, stop=True):
                nc.tensor.matmul(out_ap, lhs_ap, rhs_ap, start=start, stop=stop)

            from contextlib import nullcontext

            for it in range(NIT):
                i0 = it * ITILE
                rc = _rr_col(it)
                ri = pk_sb[:, rc:rc + ITILE]

                # ---- angular: px,py,pz matmuls + products; emitted first
                #      so it pipelines under the previous i-tile's radial.
                #      The matmuls get deferred priority so chained ang MMs
                #      never sit ahead of ready z-group MMs in the PE queue.
                def emit_ang_half(at):
                    def qs(c):
                        a0 = PK_Q + c * A + at * 128
                        return pk_sb[:, a0:a0 + 128]
                    px = pr.tile([128, ITILE], f32, tag="pr")
                    with tc.high_priority(offset=-1000):
                        mm(px[:], qs(0), ri)
                    pxs = mid.tile([128, ITILE], f32, tag="pxs")
                    # GPSIMD/Pool cannot touch PSUM on HW: DVE does all of
                    # the ang chain and the final muls.
                    nc.vector.tensor_copy(pxs[:], px[:])
                    py = pr.tile([128, ITILE], f32, tag="pr")
                    with tc.high_priority(offset=-1000):
                        mm(py[:], qs(1), ri)
                    t1 = mid.tile([128, ITILE], f32, tag="t1")
                    nc.vector.tensor_mul(t1[:], pxs[:], py[:])
                    pz = pr.tile([128, ITILE], f32, tag="pr")
                    with tc.high_priority(offset=-1000):
                        mm(pz[:], qs(2), ri)
                    a_sb = angp.tile([128, ITILE], f32, tag="ang")
                    nc.vector.tensor_mul(a_sb[:], t1[:], pz[:])
                    return a_sb

                ang = [emit_ang_half(0)]
                if it > 0:
                    ang.append(emit_ang_half(1))

                # ---- radial: z groups of r-tiles, E=exp(z) bf16,
                #      rad[at] = accumulated S^T E; final out per half.
                #      it0 uses a narrow leading group so the first
                #      activation starts one matmul earlier. ----
                if it == 0:
                    groups = [(0,), (1, 2), (3, 4), (5, 6), (7, 8), (9, 10),
                              (11,)]
                else:
                    groups = [(0, 1), (2, 3), (4, 5), (6, 7), (8, 9),
                              (10, 11)]
                rad = [None, None]
                for gidx, rts in enumerate(groups):
                    if it == 0 and gidx == 2:
                        ang.append(emit_ang_half(1))
                    gw = len(rts) * ITILE
                    with tc.high_priority():
                        z2 = zp.tile([128, 2 * ITILE], f32, tag="z")
                        for j, rt in enumerate(rts):
                            mm(z2[:, j * ITILE:(j + 1) * ITILE],
                               pk_sb[:, _w1_col(rt):_w1_col(rt) + 128], ri)
                        e2 = ep.tile([128, 2 * ITILE], bf16, tag="e")
                        nc.scalar.activation(e2[:, 0:gw], z2[:, 0:gw], EXP)
                    for j, rt in enumerate(rts):
                        at = 0 if rt < 6 else 1
                        if rt % 6 == 0:
                            rad_t = pr.tile([128, ITILE], f32, tag="pr")
                            rad[at] = rad_t
                        mm(rad[at][:], pk_sb[:, _s_col(rt):_s_col(rt) + 128],
                           e2[:, j * ITILE:(j + 1) * ITILE],
                           start=(rt % 6 == 0), stop=(rt % 6 == 5))
                        if rt % 6 == 5:
                            last = it == NIT - 1 and at == 1
                            o = op.tile([128, ITILE], f32, tag="o")
                            if last:
                                # asymmetric split of the final mul+DMA: the
                                # big piece issues on the (idle) ACT queue as
                                # soon as its mul lands; the small last piece
                                # rides SP in parallel, shrinking the tail
                                H = 352
                                nc.vector.tensor_mul(
                                    o[:, 0:H], ang[at][:, 0:H],
                                    rad[at][:, 0:H])
                                nc.scalar.dma_start(
                                    out_d[at * 128:(at + 1) * 128,
                                          i0:i0 + H], o[:, 0:H])
                                nc.vector.tensor_mul(
                                    o[:, H:], ang[at][:, H:], rad[at][:, H:])
                                nc.sync.dma_start(
                                    out_d[at * 128:(at + 1) * 128,
                                          i0 + H:i0 + ITILE], o[:, H:])
                            else:
                                nc.vector.tensor_mul(
                                    o[:], ang[at][:], rad[at][:])
                                nc.sync.dma_start(
                                    out_d[at * 128:(at + 1) * 128,
                                          i0:i0 + ITILE], o[:])

    nc.compile()
    return nc


def _consts(centers, exps, coeffs, powers):
    al = exps.astype(np.float64)
    c = coeffs.astype(np.float64)
    cen = centers.astype(np.float64)
    cc = (cen ** 2).sum(-1)
    absc = np.abs(c)
    lnc = np.where(absc > 0, np.log(np.where(absc > 0, absc, 1.0)), -1e30)
    sgn = np.sign(c)

    alf = al.reshape(-1)  # row index r = a*P + p
    w1 = np.zeros((K7, A * P))
    w1[0] = w1[1] = w1[2] = -alf
    for cd in range(3):
        w1[3 + cd] = 2.0 * alf * np.repeat(cen[:, cd], P)
    w1[6] = -alf * np.repeat(cc, P) + lnc.reshape(-1)
    w1 = np.float32(w1)

    s = np.zeros((RT, 128, 128))
    r = np.arange(A * P)
    t_of_r = r // 128
    m_of_r = (r // P) - np.where(t_of_r < RT // 2, 0, 128)
    s[t_of_r, r % 128, m_of_r] = sgn.reshape(-1)
    s2 = np.ascontiguousarray(s.transpose(1, 0, 2).reshape(128, RT * 128))

    qmat = np.zeros((3, K7, A))
    for cd in range(3):
        l = powers[:, cd].astype(np.int64)
        ccd = cen[:, cd]
        qmat[cd, cd] = (l == 2) * 1.0
        qmat[cd, 3 + cd] = (l == 1) * 1.0 + (l == 2) * (-2.0 * ccd)
        qmat[cd, 6] = (l == 0) * 1.0 + (l == 1) * (-ccd) + (l == 2) * (ccd ** 2)
    q2 = np.ascontiguousarray(
        np.float32(qmat).transpose(1, 0, 2).reshape(K7, 3 * A))

    return (_split21(w1), _bf(s2), _split21(q2))


def _basis(pos_shard):
    """Host-built [21, I] bf16 basis rows
    [xh(3), xl(3), 1, 0, sqh(3), sql(3), sqh(3), xh(3), 1], then
    zero-padded to 128 rows."""
    p64 = pos_shard.reshape(I, 3).T.astype(np.float64)      # [3, I]
    xh = _bf(p64)
    xl = _bf(p64 - xh.astype(np.float64))
    sq = p64 * p64
    sqh = _bf(sq)
    sql = _bf(sq - sqh.astype(np.float64))
    one = np.ones((1, I), ml_dtypes.bfloat16)
    zro = np.zeros((1, I), ml_dtypes.bfloat16)
    rr = np.concatenate([
        xh, xl, one, zro, sqh, sql, sqh, xh, one,
        np.zeros((128 - 21, I), ml_dtypes.bfloat16),
    ], axis=0)
    return rr


def _pack(rr, w1, q, s):
    """Assemble the [128, PK] packed input per the PK_* layout."""
    return np.ascontiguousarray(np.concatenate([
        rr[:, 0:512],          # PK_RR0
        w1[:, 0:256],          # PK_W1A
        q,                     # PK_Q
        w1[:, 256:512],        # PK_W1B
        s[:, 0:256],           # PK_SA
        s[:, 256:768],         # PK_SB
        w1[:, 512:1536],       # PK_W1C
        s[:, 768:1536],        # PK_SC
        rr[:, 512:2048],       # PK_RR1
    ], axis=1))


LAST_RESULT = None


def kernel(pos, centers, exps, coeffs, powers):
    global LAST_RESULT
    import os
    try:
        from antenv.axon_hooks import get_axon_ntff_profile_hook  # noqa: F401
    except ImportError:
        # Tracing requires the axon NTFF hook; disable rather than crash if
        # BASS_TRACE happens to be set in an environment without it.
        os.environ["BASS_NEVER_TRACE"] = "1"
    from concourse.bass_utils import run_bass_kernel_spmd

    pos = np.asarray(pos, dtype=np.float32)
    centers = np.asarray(centers, dtype=np.float32)
    exps = np.asarray(exps, dtype=np.float32)
    coeffs = np.asarray(coeffs, dtype=np.float32)
    powers = np.asarray(powers)

    if "nc" not in _CACHE:
        _CACHE["nc"] = _build_nc()
    nc = _CACHE["nc"]

    w1, s, q = _consts(centers, exps, coeffs, powers)
    in_maps = []
    for ci in range(NCORES):
        rr = _basis(pos[ci * BS:(ci + 1) * BS])
        in_maps.append({"pk": _pack(rr, w1, q, s)})

    res = run_bass_kernel_spmd(nc, in_maps, core_ids=list(range(NCORES)))
    LAST_RESULT = res
    out = np.stack(
        [res.results[ci]["out"] for ci in range(NCORES)], axis=0)  # [8, A, I]
    # [8, A, BS*NEL] -> [B, NEL, A]
    out = out.reshape(NCORES, A, BS, NEL).transpose(0, 2, 3, 1)
    return np.ascontiguousarray(out).reshape(B, NEL, A)

